# Initial kernel scaffold
#
"""CARC attention processor kernel for 8 Trainium2 NeuronCores.

Reference computation (B=1, L=4096, C=640, H=10, D=64):
    q/k/v = hidden @ Wq/Wk/Wv, split into 10 heads of 64
    k_cat = [k, 0.42*K_bg], v_cat = [v, 0.42*V_bg]   (key length 8192)
    out   = softmax(q k_cat^T / 8) v_cat, heads merged, @ Wo + bo

Sharding: queries are split 512 per core; every core computes all 10 heads
for its queries (k/v projections replicated per core — cheap relative to
attention).  Output is a disjoint row-slice per core; the host concatenates.

All matmuls run in bf16 with fp32 PSUM accumulation.  Softmax skips the
max-subtraction (scores are ~N(0,1); exp runs on ScalarE with the 1/8 scale
folded in, and the 0.42 key-side scale folded into the bg exp scale).  The
softmax denominator comes from a ones-column appended to V in the probs@V
matmul; the output-projection bias is folded in as a 65th row of Wo against
the ctx ones-column.

Heads are processed in pairs: projections compute both heads of a pair in
one matmul stream (head A on partitions 0-63, head B on 64-127), exp reads
1024-wide (two PSUM banks) per instruction, and the A/B score tiles
alternate through a shared 2-slot PSUM pool so ScalarE (the critical
engine) never starves.
"""

import numpy as np

import concourse.bass as bass
import concourse.mybir as mybir
import concourse.tile as tile

F32 = mybir.dt.float32
BF16 = mybir.dt.bfloat16
AF = mybir.ActivationFunctionType

# Problem constants (hardcoded per contract)
B, L, C = 1, 4096, 640
H, D = 10, 64
ALPHA = 0.42
N_CORES = 8
SCALE = 1.0 / np.sqrt(D)  # 0.125


class Cfg:
    def __init__(self, H=H, C=C, Lk=L, Q=L // N_CORES):
        assert C % 128 == 0 and Lk % 1024 == 0 and Q % 128 == 0 and Q <= 512
        assert H % 2 == 0
        self.H, self.C, self.Lk, self.Q = H, C, Lk, Q
        self.n_cc = C // 128      # contraction chunks for projections
        self.n_kt = Lk // 128     # key tiles per source (self / bg)
        self.n_qt = Q // 128      # query tiles of this core


def emit(nc: bass.Bass, cfg: Cfg):
    Hh, Cc, Lk, Q = cfg.H, cfg.C, cfg.Lk, cfg.Q
    n_cc, n_kt, n_qt = cfg.n_cc, cfg.n_kt, cfg.n_qt
    n_pair = Hh // 2

    hT = nc.declare_dram_parameter("hT", [Cc, Lk], F32, isOutput=False)
    hqT = nc.declare_dram_parameter("hqT", [Cc, Q], F32, isOutput=False)
    kbgT = nc.declare_dram_parameter("KbgT", [Hh, D, Lk], F32, isOutput=False)
    vbg = nc.declare_dram_parameter("Vbg", [Hh, Lk, D], F32, isOutput=False)
    wq = nc.declare_dram_parameter("Wq", [Cc, Cc], F32, isOutput=False)
    wk = nc.declare_dram_parameter("Wk", [Cc, Cc], F32, isOutput=False)
    wv = nc.declare_dram_parameter("Wv", [Cc, Cc], F32, isOutput=False)
    wob = nc.declare_dram_parameter("WoB", [Hh, D + 1, Cc], F32, isOutput=False)
    out = nc.declare_dram_parameter("out", [Q, Cc], F32, isOutput=True)

    with tile.TileContext(nc) as tc:
        with (
            tc.tile_pool(name="singles", bufs=1) as singles,
            tc.tile_pool(name="stage", bufs=1) as stage,
            tc.tile_pool(name="bgstage", bufs=2) as bgstage,
            tc.tile_pool(name="kv", bufs=2) as kv,
            tc.tile_pool(name="probs", bufs=3) as probs_pool,
            tc.tile_pool(name="outsb", bufs=2) as outsb_pool,
            tc.tile_pool(name="fin", bufs=2) as fin_pool,
            tc.tile_pool(name="ps_a", bufs=2, space="PSUM") as ps_a,
            tc.tile_pool(name="ps_sc", bufs=2, space="PSUM") as ps_sc,
            tc.tile_pool(name="ps_ctx", bufs=1, space="PSUM") as ps_ctx,
        ):
            # ---- persistent SBUF tensors ----
            hT_bf = singles.tile([128, n_cc, Lk], BF16, tag="hT_bf")
            hq_bf = singles.tile([128, n_cc, Q], BF16, tag="hq_bf")
            wq_bf = singles.tile([128, n_cc, Cc], BF16, tag="wq_bf")
            wk_bf = singles.tile([128, n_cc, Cc], BF16, tag="wk_bf")
            wv_bf = singles.tile([128, n_cc, Cc], BF16, tag="wv_bf")
            wob_bf = singles.tile([D + 1, Hh, Cc], BF16, tag="wob_bf")
            qT2_all = singles.tile([128, n_pair, Q], BF16, tag="qT2_all")
            ctxT_all = singles.tile([D + 1, Hh, Q], BF16, tag="ctxT_all")
            ones64 = singles.tile([D + 1, D], F32, tag="ones64")
            nc.vector.memset(ones64, 1.0)

            # ---- load + cast hidden (transposed) and weights ----
            for i in range(n_cc):
                st = stage.tile([128, Lk], F32, tag="stage")
                nc.sync.dma_start(out=st, in_=hT[128 * i : 128 * (i + 1), :])
                nc.vector.tensor_copy(out=hT_bf[:, i, :], in_=st)
            for i in range(n_cc):
                st = stage.tile([128, Q], F32, tag="stage")
                nc.sync.dma_start(out=st, in_=hqT[128 * i : 128 * (i + 1), :])
                nc.vector.tensor_copy(out=hq_bf[:, i, :], in_=st)
            for w_dram, w_sb in ((wq, wq_bf), (wk, wk_bf), (wv, wv_bf)):
                st = stage.tile([128, n_cc, Cc], F32, tag="stage")
                nc.sync.dma_start(
                    out=st, in_=w_dram.rearrange("(i p) n -> p i n", p=128)
                )
                nc.vector.tensor_copy(out=w_sb, in_=st)
            hh = Hh // 2
            for half in range(2):
                st = stage.tile([D + 1, hh, Cc], F32, tag="stage")
                nc.sync.dma_start(
                    out=st,
                    in_=wob[half * hh : (half + 1) * hh].rearrange("h p n -> p h n"),
                )
                nc.vector.tensor_copy(
                    out=wob_bf[:, half * hh : (half + 1) * hh, :], in_=st
                )

            # ---- q projections, head pairs packed on partitions ----
            for p in range(n_pair):
                ps = ps_a.tile([128, Q], F32, tag="ps_a", name=f"qps{p}")
                for i in range(n_cc):
                    nc.tensor.matmul(
                        ps,
                        lhsT=wq_bf[:, i, 128 * p : 128 * (p + 1)],
                        rhs=hq_bf[:, i, :],
                        start=(i == 0),
                        stop=(i == n_cc - 1),
                    )
                nc.vector.tensor_copy(out=qT2_all[:, p, :], in_=ps)

            # ---- per head-pair: project k/v, load bg kv, attention ----
            for p in range(n_pair):
                kT2 = kv.tile([128, Lk], BF16, tag="kT")
                v2t = kv.tile([128, n_kt, 2 * (D + 1)], BF16, tag="v")
                kbg2 = kv.tile([128, Lk], BF16, tag="kbg")
                vbg2 = kv.tile([128, n_kt, 2 * (D + 1)], BF16, tag="vbg")

                # kT2 = (hidden @ Wk_pair)^T, head A on partitions 0-63
                for t in range(Lk // 512):
                    ps = ps_a.tile([128, 512], F32, tag="ps_a", name=f"kps{p}{t}")
                    for i in range(n_cc):
                        nc.tensor.matmul(
                            ps,
                            lhsT=wk_bf[:, i, 128 * p : 128 * (p + 1)],
                            rhs=hT_bf[:, i, 512 * t : 512 * (t + 1)],
                            start=(i == 0),
                            stop=(i == n_cc - 1),
                        )
                    nc.vector.tensor_copy(
                        out=kT2[:, 512 * t : 512 * (t + 1)], in_=ps
                    )
                # v natural [keys, D] for both heads (+ones cols)
                for kt in range(n_kt):
                    ps = ps_a.tile([128, 128], F32, tag="ps_a", name=f"vps{p}{kt}")
                    for i in range(n_cc):
                        nc.tensor.matmul(
                            ps,
                            lhsT=hT_bf[:, i, 128 * kt : 128 * (kt + 1)],
                            rhs=wv_bf[:, i, 128 * p : 128 * (p + 1)],
                            start=(i == 0),
                            stop=(i == n_cc - 1),
                        )
                    nc.vector.tensor_copy(out=v2t[:, kt, 0:D], in_=ps[:, 0:D])
                    nc.vector.tensor_copy(
                        out=v2t[:, kt, D + 1 : 2 * D + 1], in_=ps[:, D : 2 * D]
                    )
                nc.vector.memset(v2t[:, :, D : D + 1], 1.0)
                nc.vector.memset(v2t[:, :, 2 * D + 1 : 2 * D + 2], 1.0)

                # bg K (transposed) and bg V (scaled by ALPHA at load),
                # staged in 1/4 pieces to bound SBUF staging space
                for p4 in range(4):
                    lw = Lk // 4
                    tw = n_kt // 4
                    st = bgstage.tile([128, lw], F32, tag="kbg_st", name=f"kst{p}{p4}")
                    nc.sync.dma_start(
                        out=st[0:D, :], in_=kbgT[2 * p, :, lw * p4 : lw * (p4 + 1)]
                    )
                    nc.sync.dma_start(
                        out=st[D : 2 * D, :],
                        in_=kbgT[2 * p + 1, :, lw * p4 : lw * (p4 + 1)],
                    )
                    nc.vector.tensor_copy(
                        out=kbg2[:, lw * p4 : lw * (p4 + 1)], in_=st
                    )
                    st2 = bgstage.tile(
                        [128, tw, 2 * D], F32, tag="vbg_st", name=f"vst{p}{p4}"
                    )
                    for hi in range(2):
                        nc.sync.dma_start(
                            out=st2[:, :, D * hi : D * (hi + 1)],
                            in_=vbg[
                                2 * p + hi, lw * p4 : lw * (p4 + 1), :
                            ].rearrange("(kt q) d -> q kt d", q=128),
                        )
                        nc.vector.tensor_scalar_mul(
                            vbg2[
                                :,
                                tw * p4 : tw * (p4 + 1),
                                (D + 1) * hi : (D + 1) * hi + D,
                            ],
                            st2[:, :, D * hi : D * (hi + 1)],
                            ALPHA,
                        )
                nc.vector.memset(vbg2[:, :, D : D + 1], 1.0)
                nc.vector.memset(vbg2[:, :, 2 * D + 1 : 2 * D + 2], 1.0)

                # ---- attention for the pair ----
                # ctx accumulators: head A in PSUM bank 0, head B in bank 1
                ctx2 = ps_ctx.tile([D + 1, 2, 512], F32, tag="ctx", name=f"ctx{p}")
                n_k2 = n_kt // 2
                for src in range(2):  # 0=self keys, 1=bg keys
                    kk = kT2 if src == 0 else kbg2
                    vv = v2t if src == 0 else vbg2
                    e_scale = SCALE if src == 0 else SCALE * ALPHA
                    for k2 in range(n_k2):
                        first = src == 0 and k2 == 0
                        last = src == 1 and k2 == n_k2 - 1
                        # QK for heads A/B issued back-to-back per key tile:
                        # distinct PE row groups (tile_position) let the two
                        # K=64 matmuls stream concurrently.
                        scs = []
                        for hi in range(2):
                            scs.append(
                                ps_sc.tile(
                                    [128, 2, Q],
                                    F32,
                                    tag="sc",
                                    name=f"sc{p}{src}{k2}{hi}",
                                )
                            )
                        for j in range(2):
                            kt = 2 * k2 + j
                            for hi in range(2):
                                nc.tensor.matmul(
                                    scs[hi][:, j, :],
                                    lhsT=kk[
                                        D * hi : D * (hi + 1),
                                        128 * kt : 128 * (kt + 1),
                                    ],
                                    rhs=qT2_all[D * hi : D * (hi + 1), p, :],
                                    start=True,
                                    stop=True,
                                    tile_position=(D * hi, 0),
                                )
                        prs = []
                        for hi in range(2):
                            pr = probs_pool.tile(
                                [128, 2, Q], BF16, tag="pr", name=f"pr{p}{src}{k2}{hi}"
                            )
                            nc.scalar.activation(pr, scs[hi], AF.Exp, scale=e_scale)
                            prs.append(pr)
                        for hi in range(2):
                            for j in range(2):
                                kt = 2 * k2 + j
                                nc.tensor.matmul(
                                    ctx2[:, hi, 0:Q],
                                    lhsT=vv[
                                        :, kt, (D + 1) * hi : (D + 1) * (hi + 1)
                                    ],
                                    rhs=prs[hi][:, j, :],
                                    start=(first and j == 0),
                                    stop=(last and j == 1),
                                )
                # normalize: denom row (partition 64) -> broadcast over the
                # 64 d-partitions via a K=1 fp32 matmul, then recip + mul
                for hi in range(2):
                    h = 2 * p + hi
                    fin = fin_pool.tile([D + 1, Q], F32, tag="fin", name=f"fin{h}")
                    nc.vector.tensor_copy(
                        out=fin[D : D + 1, :], in_=ctx2[D : D + 1, hi, 0:Q]
                    )
                    bc = ps_a.tile([D, Q], F32, tag="ps_a", name=f"bc{h}")
                    nc.tensor.matmul(
                        bc,
                        lhsT=ones64[D : D + 1, :],
                        rhs=fin[D : D + 1, :],
                        start=True,
                        stop=True,
                        tile_position=(D, 0),
                    )
                    nc.vector.reciprocal(fin[0:D, :], bc)
                    nc.vector.tensor_mul(
                        ctxT_all[0:D, h, :], ctx2[0:D, hi, 0:Q], fin[0:D, :]
                    )
                    nc.vector.memset(ctxT_all[D : D + 1, h, :], 1.0)

            # ---- output projection: out[qt] = sum_h ctxT_h^T @ WoB_h ----
            for qt in range(n_qt):
                o_sb = outsb_pool.tile([128, Cc], F32, tag="o_sb")
                for n0 in range(0, Cc, 512):
                    nw = min(512, Cc - n0)
                    ps = ps_sc.tile([128, 2, Q], F32, tag="sc", name=f"ops{qt}{n0}")
                    for h in range(Hh):
                        nc.tensor.matmul(
                            ps[:, 0, 0:nw],
                            lhsT=ctxT_all[:, h, 128 * qt : 128 * (qt + 1)],
                            rhs=wob_bf[:, h, n0 : n0 + nw],
                            start=(h == 0),
                            stop=(h == Hh - 1),
                        )
                    nc.vector.tensor_copy(out=o_sb[:, n0 : n0 + nw], in_=ps[:, 0, 0:nw])
                nc.sync.dma_start(
                    out=out[128 * qt : 128 * (qt + 1), :], in_=o_sb
                )
    return nc


def split_waits(nc, limit=1):
    """This container's walrus rejects >limit sync waits per instruction;
    hoist excess waits onto standalone EventSemaphore instructions."""
    cnt = 0
    for f in nc.m.functions:
        for bb in f.blocks:
            fixed = []
            for inst in bb.instructions:
                si = inst.sync_info
                if si is not None and len(si.on_wait) > limit:
                    waits = list(si.on_wait)
                    extra, keep = waits[:-limit], waits[-limit:]
                    for w in extra:
                        cnt += 1
                        ev = mybir.InstEventSemaphore(
                            name=f"I-waitsplit-{cnt}", ins=[], outs=[]
                        )
                        ev.engine = inst.engine
                        ev.sync_info = mybir.SyncInfo(on_wait=[w], on_update=[])
                        nc.register_instruction(ev)
                        fixed.append(ev)
                    si.on_wait = keep
                fixed.append(inst)
            bb.instructions[:] = fixed
    return cnt


def build_bass(cfg: Cfg | None = None):
    cfg = cfg or Cfg()
    nc = bass.Bass()
    emit(nc, cfg)
    split_waits(nc)
    return nc


def make_in_maps(hidden_states, K_bg, V_bg, Wq, Wk, Wv, Wo, bo):
    hT = np.ascontiguousarray(np.asarray(hidden_states, np.float32)[0].T)
    KbgT = np.ascontiguousarray(np.asarray(K_bg, np.float32).transpose(0, 2, 1))
    WoB = np.zeros((H, D + 1, C), np.float32)
    WoB[:, :D, :] = np.asarray(Wo, np.float32).reshape(H, D, C)
    WoB[0, D, :] = np.asarray(bo, np.float32)
    common = {
        "hT": hT,
        "KbgT": KbgT,
        "Vbg": np.ascontiguousarray(np.asarray(V_bg, np.float32)),
        "Wq": np.asarray(Wq, np.float32),
        "Wk": np.asarray(Wk, np.float32),
        "Wv": np.asarray(Wv, np.float32),
        "WoB": WoB,
    }
    qs = L // N_CORES
    return [
        dict(common, hqT=np.ascontiguousarray(hT[:, qs * c : qs * (c + 1)]))
        for c in range(N_CORES)
    ]


_NC_CACHE = {}


def kernel(hidden_states, K_bg, V_bg, Wq, Wk, Wv, Wo, bo):
    if "nc" not in _NC_CACHE:
        _NC_CACHE["nc"] = build_bass()
    nc = _NC_CACHE["nc"]
    in_maps = make_in_maps(hidden_states, K_bg, V_bg, Wq, Wk, Wv, Wo, bo)
    from concourse import bass2jax

    results = bass2jax.run_bass_via_pjrt(nc, in_maps, n_cores=N_CORES)
    out = np.concatenate([results[c]["out"] for c in range(N_CORES)], axis=0)
    return out.reshape(B, L, C)



# revision 1
# speedup vs baseline: 1.0715x; 1.0715x over previous
"""CARC attention processor kernel for 8 Trainium2 NeuronCores.

Reference computation (B=1, L=4096, C=640, H=10, D=64):
    q/k/v = hidden @ Wq/Wk/Wv, split into 10 heads of 64
    k_cat = [k, 0.42*K_bg], v_cat = [v, 0.42*V_bg]   (key length 8192)
    out   = softmax(q k_cat^T / 8) v_cat, heads merged, @ Wo + bo

Sharding: queries are split 512 per core; every core computes all 10 heads
for its queries (k/v projections replicated per core — cheap relative to
attention).  Output is a disjoint row-slice per core; the host concatenates.

All matmuls run in bf16 with fp32 PSUM accumulation.  Softmax skips the
max-subtraction (scores are ~N(0,1); exp runs on ScalarE with the 1/8 scale
folded in, and the 0.42 key-side scale folded into the bg exp scale).  The
softmax denominator comes from a ones-column appended to V in the probs@V
matmul; the output-projection bias is folded in as a 65th row of Wo against
the ctx ones-column.

Heads are processed in pairs: projections compute both heads of a pair in
one matmul stream (head A on partitions 0-63, head B on 64-127), exp reads
1024-wide (two PSUM banks) per instruction, and the A/B score tiles
alternate through a shared 2-slot PSUM pool so ScalarE (the critical
engine) never starves.
"""

import numpy as np

import concourse.bass as bass
import concourse.mybir as mybir
import concourse.tile as tile

F32 = mybir.dt.float32
BF16 = mybir.dt.bfloat16
AF = mybir.ActivationFunctionType

# Problem constants (hardcoded per contract)
B, L, C = 1, 4096, 640
H, D = 10, 64
ALPHA = 0.42
N_CORES = 8
SCALE = 1.0 / np.sqrt(D)  # 0.125


class Cfg:
    def __init__(self, H=H, C=C, Lk=L, Q=L // N_CORES):
        assert C % 128 == 0 and Lk % 1024 == 0 and Q % 128 == 0 and Q <= 512
        assert H % 2 == 0
        self.H, self.C, self.Lk, self.Q = H, C, Lk, Q
        self.n_cc = C // 128      # contraction chunks for projections
        self.n_kt = Lk // 128     # key tiles per source (self / bg)
        self.n_qt = Q // 128      # query tiles of this core


def emit(nc: bass.Bass, cfg: Cfg):
    Hh, Cc, Lk, Q = cfg.H, cfg.C, cfg.Lk, cfg.Q
    n_cc, n_kt, n_qt = cfg.n_cc, cfg.n_kt, cfg.n_qt
    n_pair = Hh // 2

    hT = nc.declare_dram_parameter("hT", [Cc, Lk], F32, isOutput=False)
    hqT = nc.declare_dram_parameter("hqT", [Cc, Q], F32, isOutput=False)
    kbgT = nc.declare_dram_parameter("KbgT", [Hh, D, Lk], F32, isOutput=False)
    vbg = nc.declare_dram_parameter("Vbg", [Hh, Lk, D], F32, isOutput=False)
    wq = nc.declare_dram_parameter("Wq", [Cc, Cc], F32, isOutput=False)
    wk = nc.declare_dram_parameter("Wk", [Cc, Cc], F32, isOutput=False)
    wv = nc.declare_dram_parameter("Wv", [Cc, Cc], F32, isOutput=False)
    wob = nc.declare_dram_parameter("WoB", [Hh, D + 1, Cc], F32, isOutput=False)
    out = nc.declare_dram_parameter("out", [Q, Cc], F32, isOutput=True)

    with tile.TileContext(nc) as tc:
        with (
            tc.tile_pool(name="singles", bufs=1) as singles,
            tc.tile_pool(name="stage", bufs=1) as stage,
            tc.tile_pool(name="bgstage", bufs=2) as bgstage,
            tc.tile_pool(name="kv", bufs=2) as kv,
            tc.tile_pool(name="probs", bufs=3) as probs_pool,
            tc.tile_pool(name="outsb", bufs=2) as outsb_pool,
            tc.tile_pool(name="fin", bufs=2) as fin_pool,
            tc.tile_pool(name="ps_a", bufs=2, space="PSUM") as ps_a,
            tc.tile_pool(name="ps_sc", bufs=2, space="PSUM") as ps_sc,
            tc.tile_pool(name="ps_ctx", bufs=1, space="PSUM") as ps_ctx,
        ):
            # ---- persistent SBUF tensors ----
            hT_bf = singles.tile([128, n_cc, Lk], BF16, tag="hT_bf")
            hq_bf = singles.tile([128, n_cc, Q], BF16, tag="hq_bf")
            wq_bf = singles.tile([128, n_cc, Cc], BF16, tag="wq_bf")
            wk_bf = singles.tile([128, n_cc, Cc], BF16, tag="wk_bf")
            wv_bf = singles.tile([128, n_cc, Cc], BF16, tag="wv_bf")
            wob_bf = singles.tile([D + 1, Hh, Cc], BF16, tag="wob_bf")
            qT2_all = singles.tile([128, n_pair, Q], BF16, tag="qT2_all")
            ctxT_all = singles.tile([D + 1, Hh, Q], BF16, tag="ctxT_all")
            ones64 = singles.tile([D + 1, D], F32, tag="ones64")
            nc.vector.memset(ones64, 1.0)

            # ---- load + cast hidden (transposed) and weights ----
            for i in range(n_cc):
                st = stage.tile([128, Lk], F32, tag="stage")
                nc.sync.dma_start(out=st, in_=hT[128 * i : 128 * (i + 1), :])
                nc.vector.tensor_copy(out=hT_bf[:, i, :], in_=st)
            for i in range(n_cc):
                st = stage.tile([128, Q], F32, tag="stage")
                nc.sync.dma_start(out=st, in_=hqT[128 * i : 128 * (i + 1), :])
                nc.vector.tensor_copy(out=hq_bf[:, i, :], in_=st)
            for w_dram, w_sb in ((wq, wq_bf), (wk, wk_bf), (wv, wv_bf)):
                st = stage.tile([128, n_cc, Cc], F32, tag="stage")
                nc.sync.dma_start(
                    out=st, in_=w_dram.rearrange("(i p) n -> p i n", p=128)
                )
                nc.vector.tensor_copy(out=w_sb, in_=st)
            hh = Hh // 2
            for half in range(2):
                st = stage.tile([D + 1, hh, Cc], F32, tag="stage")
                nc.sync.dma_start(
                    out=st,
                    in_=wob[half * hh : (half + 1) * hh].rearrange("h p n -> p h n"),
                )
                nc.vector.tensor_copy(
                    out=wob_bf[:, half * hh : (half + 1) * hh, :], in_=st
                )

            # ---- q projections, head pairs packed on partitions ----
            for p in range(n_pair):
                ps = ps_a.tile([128, Q], F32, tag="ps_a", name=f"qps{p}")
                for i in range(n_cc):
                    nc.tensor.matmul(
                        ps,
                        lhsT=wq_bf[:, i, 128 * p : 128 * (p + 1)],
                        rhs=hq_bf[:, i, :],
                        start=(i == 0),
                        stop=(i == n_cc - 1),
                    )
                nc.vector.tensor_copy(out=qT2_all[:, p, :], in_=ps)

            # ---- per head-pair: project k/v, load bg kv, attention ----
            for p in range(n_pair):
                kT2 = kv.tile([128, Lk], BF16, tag="kT")
                v2t = kv.tile([128, n_kt, 2 * (D + 1)], BF16, tag="v")
                kbg2 = kv.tile([128, Lk], BF16, tag="kbg")
                vbg2 = kv.tile([128, n_kt, 2 * (D + 1)], BF16, tag="vbg")

                # kT2 = (hidden @ Wk_pair)^T, head A on partitions 0-63
                for t in range(Lk // 512):
                    ps = ps_a.tile([128, 512], F32, tag="ps_a", name=f"kps{p}{t}")
                    for i in range(n_cc):
                        nc.tensor.matmul(
                            ps,
                            lhsT=wk_bf[:, i, 128 * p : 128 * (p + 1)],
                            rhs=hT_bf[:, i, 512 * t : 512 * (t + 1)],
                            start=(i == 0),
                            stop=(i == n_cc - 1),
                        )
                    nc.vector.tensor_copy(
                        out=kT2[:, 512 * t : 512 * (t + 1)], in_=ps
                    )
                # v natural [keys, D] for both heads (+ones cols)
                for kt in range(n_kt):
                    ps = ps_a.tile([128, 128], F32, tag="ps_a", name=f"vps{p}{kt}")
                    for i in range(n_cc):
                        nc.tensor.matmul(
                            ps,
                            lhsT=hT_bf[:, i, 128 * kt : 128 * (kt + 1)],
                            rhs=wv_bf[:, i, 128 * p : 128 * (p + 1)],
                            start=(i == 0),
                            stop=(i == n_cc - 1),
                        )
                    nc.vector.tensor_copy(out=v2t[:, kt, 0:D], in_=ps[:, 0:D])
                    nc.vector.tensor_copy(
                        out=v2t[:, kt, D + 1 : 2 * D + 1], in_=ps[:, D : 2 * D]
                    )
                nc.vector.memset(v2t[:, :, D : D + 1], 1.0)
                nc.vector.memset(v2t[:, :, 2 * D + 1 : 2 * D + 2], 1.0)

                # bg K (transposed) and bg V (scaled by ALPHA at load),
                # staged in 1/4 pieces to bound SBUF staging space
                for p4 in range(4):
                    lw = Lk // 4
                    tw = n_kt // 4
                    st = bgstage.tile([128, lw], F32, tag="kbg_st", name=f"kst{p}{p4}")
                    nc.sync.dma_start(
                        out=st[0:D, :], in_=kbgT[2 * p, :, lw * p4 : lw * (p4 + 1)]
                    )
                    nc.sync.dma_start(
                        out=st[D : 2 * D, :],
                        in_=kbgT[2 * p + 1, :, lw * p4 : lw * (p4 + 1)],
                    )
                    nc.vector.tensor_copy(
                        out=kbg2[:, lw * p4 : lw * (p4 + 1)], in_=st
                    )
                    st2 = bgstage.tile(
                        [128, tw, 2 * D], F32, tag="vbg_st", name=f"vst{p}{p4}"
                    )
                    for hi in range(2):
                        nc.sync.dma_start(
                            out=st2[:, :, D * hi : D * (hi + 1)],
                            in_=vbg[
                                2 * p + hi, lw * p4 : lw * (p4 + 1), :
                            ].rearrange("(kt q) d -> q kt d", q=128),
                        )
                        nc.vector.tensor_scalar_mul(
                            vbg2[
                                :,
                                tw * p4 : tw * (p4 + 1),
                                (D + 1) * hi : (D + 1) * hi + D,
                            ],
                            st2[:, :, D * hi : D * (hi + 1)],
                            ALPHA,
                        )
                nc.vector.memset(vbg2[:, :, D : D + 1], 1.0)
                nc.vector.memset(vbg2[:, :, 2 * D + 1 : 2 * D + 2], 1.0)

                # ---- attention for the pair ----
                # ctx accumulators: head A in PSUM bank 0, head B in bank 1
                ctx2 = ps_ctx.tile([D + 1, 2, 512], F32, tag="ctx", name=f"ctx{p}")
                n_k2 = n_kt // 2
                for src in range(2):  # 0=self keys, 1=bg keys
                    kk = kT2 if src == 0 else kbg2
                    vv = v2t if src == 0 else vbg2
                    e_scale = SCALE if src == 0 else SCALE * ALPHA
                    for k2 in range(n_k2):
                        first = src == 0 and k2 == 0
                        last = src == 1 and k2 == n_k2 - 1
                        # QK for heads A/B issued back-to-back per key tile:
                        # distinct PE row groups (tile_position) let the two
                        # K=64 matmuls stream concurrently.
                        scs = []
                        for hi in range(2):
                            scs.append(
                                ps_sc.tile(
                                    [128, 2, Q],
                                    F32,
                                    tag="sc",
                                    name=f"sc{p}{src}{k2}{hi}",
                                )
                            )
                        for j in range(2):
                            kt = 2 * k2 + j
                            for hi in range(2):
                                nc.tensor.matmul(
                                    scs[hi][:, j, :],
                                    lhsT=kk[
                                        D * hi : D * (hi + 1),
                                        128 * kt : 128 * (kt + 1),
                                    ],
                                    rhs=qT2_all[D * hi : D * (hi + 1), p, :],
                                    start=True,
                                    stop=True,
                                    tile_position=(D * hi, 0),
                                )
                        prs = []
                        for hi in range(2):
                            pr = probs_pool.tile(
                                [128, 2, Q], BF16, tag="pr", name=f"pr{p}{src}{k2}{hi}"
                            )
                            nc.scalar.activation(pr, scs[hi], AF.Exp, scale=e_scale)
                            prs.append(pr)
                        for hi in range(2):
                            for j in range(2):
                                kt = 2 * k2 + j
                                nc.tensor.matmul(
                                    ctx2[:, hi, 0:Q],
                                    lhsT=vv[
                                        :, kt, (D + 1) * hi : (D + 1) * (hi + 1)
                                    ],
                                    rhs=prs[hi][:, j, :],
                                    start=(first and j == 0),
                                    stop=(last and j == 1),
                                )
                # normalize: denom row (partition 64) -> broadcast over the
                # 64 d-partitions via a K=1 fp32 matmul, then recip + mul
                for hi in range(2):
                    h = 2 * p + hi
                    fin = fin_pool.tile([D + 1, Q], F32, tag="fin", name=f"fin{h}")
                    nc.vector.tensor_copy(
                        out=fin[D : D + 1, :], in_=ctx2[D : D + 1, hi, 0:Q]
                    )
                    bc = ps_a.tile([D, Q], F32, tag="ps_a", name=f"bc{h}")
                    nc.tensor.matmul(
                        bc,
                        lhsT=ones64[D : D + 1, :],
                        rhs=fin[D : D + 1, :],
                        start=True,
                        stop=True,
                        tile_position=(D, 0),
                    )
                    nc.vector.reciprocal(fin[0:D, :], bc)
                    nc.vector.tensor_mul(
                        ctxT_all[0:D, h, :], ctx2[0:D, hi, 0:Q], fin[0:D, :]
                    )
                    nc.vector.memset(ctxT_all[D : D + 1, h, :], 1.0)

            # ---- output projection: out[qt] = sum_h ctxT_h^T @ WoB_h ----
            for qt in range(n_qt):
                o_sb = outsb_pool.tile([128, Cc], F32, tag="o_sb")
                for n0 in range(0, Cc, 512):
                    nw = min(512, Cc - n0)
                    ps = ps_sc.tile([128, 2, Q], F32, tag="sc", name=f"ops{qt}{n0}")
                    for h in range(Hh):
                        nc.tensor.matmul(
                            ps[:, 0, 0:nw],
                            lhsT=ctxT_all[:, h, 128 * qt : 128 * (qt + 1)],
                            rhs=wob_bf[:, h, n0 : n0 + nw],
                            start=(h == 0),
                            stop=(h == Hh - 1),
                        )
                    nc.vector.tensor_copy(out=o_sb[:, n0 : n0 + nw], in_=ps[:, 0, 0:nw])
                nc.sync.dma_start(
                    out=out[128 * qt : 128 * (qt + 1), :], in_=o_sb
                )
    return nc


def split_waits(nc, limit=1):
    """This container's walrus rejects >limit sync waits per instruction;
    hoist excess waits onto standalone EventSemaphore instructions."""
    cnt = 0
    for f in nc.m.functions:
        for bb in f.blocks:
            fixed = []
            for inst in bb.instructions:
                si = inst.sync_info
                if si is not None and len(si.on_wait) > limit:
                    waits = list(si.on_wait)
                    extra, keep = waits[:-limit], waits[-limit:]
                    for w in extra:
                        cnt += 1
                        ev = mybir.InstEventSemaphore(
                            name=f"I-waitsplit-{cnt}", ins=[], outs=[]
                        )
                        ev.engine = inst.engine
                        ev.sync_info = mybir.SyncInfo(on_wait=[w], on_update=[])
                        nc.register_instruction(ev)
                        fixed.append(ev)
                    si.on_wait = keep
                fixed.append(inst)
            bb.instructions[:] = fixed
    return cnt


def build_bass(cfg: Cfg | None = None):
    cfg = cfg or Cfg()
    nc = bass.Bass()
    emit(nc, cfg)
    split_waits(nc)
    return nc


def make_in_maps(hidden_states, K_bg, V_bg, Wq, Wk, Wv, Wo, bo):
    hT = np.ascontiguousarray(np.asarray(hidden_states, np.float32)[0].T)
    KbgT = np.ascontiguousarray(np.asarray(K_bg, np.float32).transpose(0, 2, 1))
    WoB = np.zeros((H, D + 1, C), np.float32)
    WoB[:, :D, :] = np.asarray(Wo, np.float32).reshape(H, D, C)
    WoB[0, D, :] = np.asarray(bo, np.float32)
    common = {
        "hT": hT,
        "KbgT": KbgT,
        "Vbg": np.ascontiguousarray(np.asarray(V_bg, np.float32)),
        "Wq": np.asarray(Wq, np.float32),
        "Wk": np.asarray(Wk, np.float32),
        "Wv": np.asarray(Wv, np.float32),
        "WoB": WoB,
    }
    qs = L // N_CORES
    return [
        dict(common, hqT=np.ascontiguousarray(hT[:, qs * c : qs * (c + 1)]))
        for c in range(N_CORES)
    ]


_NC_CACHE = {}


def kernel(hidden_states, K_bg, V_bg, Wq, Wk, Wv, Wo, bo):
    if "nc" not in _NC_CACHE:
        _NC_CACHE["nc"] = build_bass()
    nc = _NC_CACHE["nc"]
    in_maps = make_in_maps(hidden_states, K_bg, V_bg, Wq, Wk, Wv, Wo, bo)
    from concourse import bass2jax

    results = bass2jax.run_bass_via_pjrt(nc, in_maps, n_cores=N_CORES)
    out = np.concatenate([results[c]["out"] for c in range(N_CORES)], axis=0)
    return out.reshape(B, L, C)

